# revision 1
# baseline (speedup 1.0000x reference)
"""NonLocalBlock (B=4, C=256, H=W=64) Trainium2 Bass kernel.

Sharding: 8 cores = 4 batch elements x 2 query-row shards of 2048 rows.
Each core receives its batch element's x rotated along N so that its
query rows are columns [0, 2048) -- the program is identical on every
core (pure SPMD), only the data differs.

Per-core pipeline:
  A) 1x1-conv projections on the PE:
       theta[d, nq] (queries, f32r), phi[d, m] (keys, f32r),
       gT[m, d] (values, transposed chunk-major layout, fp16)
  B) attention, streamed per 512-column query block:
       S^T[m-chunk, nblk] = phi_chunk x theta   (PE, f32r logits)
       P^T = exp(S^T / sqrt(D))                 (ScalarE, -> fp16)
       y[d, nblk]  += gT_chunk.T @ P^T          (PE fp16, PSUM accum)
       r[*, nblk]  += ones.T @ P^T              (PE fp16; softmax denom,
                                                 replicated over partitions)
       rho = exp(-ln(r))                        (ScalarE; joint Exp+Ln
                                                 table set, no reloads)
       out_norm[c, nblk] = (wo.T @ y) * rho     (PE f32r out-conv, then
                                                 one DVE stt per chunk
                                                 with accum -> s1)
       s2 += sum(out_norm^2)                    (DVE stt accum)
  C) BatchNorm (training mode, batch stats): AllReduce [s1|s2] across
     all 8 cores, mean/var/rstd on-chip, then out = x + a*out + b with
     a = gamma*rstd, b = beta - a*mean.  (out_conv bias cancels in
     training-mode BN and is skipped entirely.)

float32r streams at 2 cycles/row on HW; it is kept for the
precision-critical logit path (theta/phi projections + S^T).
Post-softmax paths (P, g, ones) are fp16 -- rounding errors average
out over the 4096-key softmax sum.
"""

import math
import os

import numpy as np

import concourse.bass as bass
import concourse.mybir as mybir
import concourse.tile as tile
from concourse import bacc
from concourse.bass_utils import run_bass_kernel_spmd

# Problem constants (hardcoded per contract).
B, C, HGT, WID = 4, 256, 64, 64
N = HGT * WID            # 4096 spatial positions
D = C // 2               # 128 inner channels
P = 128                  # SBUF partitions
NCORES = 8
SPLIT = NCORES // B      # query shards per batch element
NQ = N // SPLIT          # 2048 query rows per core
CB = C // P              # 2 channel chunks
MCH = N // P             # 32 key chunks
NBLK = 512               # query block (one PSUM bank)
NB = NQ // NBLK          # 4 blocks
EPS = 1e-5
SCALE = 1.0 / math.sqrt(D)
NSAMP = float(B * N)     # BN sample count per channel

F32 = mybir.dt.float32
F32R = mybir.dt.float32r
F16 = mybir.dt.float16

AF = mybir.ActivationFunctionType
ALU = mybir.AluOpType
AX = mybir.AxisListType

_CACHED_NC = None


def _compile_with_joint_act_tables(nc):
    """Run bacc passes with Exp/Ln resolving to the joint table set.

    The default per-function chooser picks `exp_and_others` for Exp and
    `natural_log` for Ln, causing ~1.3us table reloads whenever the two
    alternate.  Emptying those two sets (preserving dict order, so the
    walrus set ids stay aligned) forces both functions onto
    `natural_log_exp_and_others`.
    """
    real = bacc.get_activation_tables

    def patched(arch):
        t = dict(real(arch))
        for k in ("exp_and_others", "natural_log"):
            if k in t:
                t[k] = type(t[k])()
        return t

    bacc.get_activation_tables = patched
    try:
        nc.compile()
    finally:
        bacc.get_activation_tables = real


def _build_nc():
    nc = bacc.Bacc("TRN2", target_bir_lowering=False, debug=False,
                   num_devices=NCORES)

    x_d = nc.dram_tensor("x", [C, N], F32R, kind="ExternalInput")
    # f32r weights: wq|wk (2*C cols) then wo (C cols)
    wp_d = nc.dram_tensor("wpack", [P, 3 * C], F32R, kind="ExternalInput")
    # fp16 value weights: wv (C cols)
    wv_d = nc.dram_tensor("wvb", [P, C], F16, kind="ExternalInput")
    # small fp32 constants: bq|bk|bv|gam|bet
    cp_d = nc.dram_tensor("cpack", [P, 1 + 1 + P + CB + CB], F32,
                          kind="ExternalInput")
    out_d = nc.dram_tensor("out", [C, NQ], F32, kind="ExternalOutput")

    with tile.TileContext(nc) as tc:
        with (
            tc.tile_pool(name="consts", bufs=1) as consts,
            tc.tile_pool(name="bigs", bufs=1) as bigs,
            tc.tile_pool(name="ptp", bufs=3) as ptp,
            tc.tile_pool(name="work", bufs=2) as work,
            tc.tile_pool(name="ps", bufs=2, space="PSUM") as ps,
            tc.tile_pool(name="dram", bufs=1, space="DRAM") as dram,
        ):
            # ---- constant / weight loads (gpsimd -> one DMASW0 sem) ----
            wpack = consts.tile([P, 3 * C], F32R)
            wvb = consts.tile([P, C], F16)
            cpack = consts.tile([P, 1 + 1 + P + CB + CB], F32)
            nc.gpsimd.dma_start(wpack[:], wp_d[:])
            nc.gpsimd.dma_start(wvb[:], wv_d[:])
            nc.gpsimd.dma_start(cpack[:], cp_d[:])
            wq = wpack[:, 0 * C:1 * C]
            wk = wpack[:, 1 * C:2 * C]
            wo = wpack[:, 2 * C:3 * C]
            bq = cpack[:, 0:1]
            bk = cpack[:, 1:2]
            bv = cpack[:, 2:2 + P]
            gam = cpack[:, 2 + P:2 + P + CB]
            bet = cpack[:, 2 + P + CB:2 + P + 2 * CB]
            ones = consts.tile([P, P], F16)
            nc.vector.memset(ones[:], 1.0)

            # ---- x load (f32r), plus fp16 cast for the value path ----
            XCH = 4
            xs = [bigs.tile([P, N], F32R, name=f"x{cb}", tag=f"x{cb}")
                  for cb in range(CB)]
            for k in range(XCH):
                ksl = slice(k * (N // XCH), (k + 1) * (N // XCH))
                for cb in range(CB):
                    nc.gpsimd.dma_start(xs[cb][:, ksl],
                                        x_d[cb * P:(cb + 1) * P, ksl])
            xb16 = [bigs.tile([P, N], F16, name=f"xb{cb}", tag=f"xb{cb}")
                    for cb in range(CB)]
            for cb in range(CB):
                for k in range(XCH):
                    ksl = slice(k * (N // XCH), (k + 1) * (N // XCH))
                    nc.vector.tensor_copy(xb16[cb][:, ksl], xs[cb][:, ksl])

            # ---- phase A: projections ----
            theta = bigs.tile([P, NQ], F32R, tag="theta")
            phi = bigs.tile([P, N], F32R, tag="phi")
            gT = bigs.tile([P, N], F16, tag="gT")  # [m%128, 128*mc + d]

            for j in range(NQ // NBLK):
                sl = slice(j * NBLK, (j + 1) * NBLK)
                pt = ps.tile([P, NBLK], F32, tag="ps_s")
                for cb in range(CB):
                    nc.tensor.matmul(
                        pt[:], wq[:, cb * P:(cb + 1) * P], xs[cb][:, sl],
                        start=(cb == 0), stop=(cb == CB - 1))
                nc.vector.tensor_scalar_add(theta[:, sl], pt[:], bq[:])
            for j in range(N // NBLK):
                sl = slice(j * NBLK, (j + 1) * NBLK)
                pt = ps.tile([P, NBLK], F32, tag="ps_s")
                for cb in range(CB):
                    nc.tensor.matmul(
                        pt[:], wk[:, cb * P:(cb + 1) * P], xs[cb][:, sl],
                        start=(cb == 0), stop=(cb == CB - 1))
                nc.vector.tensor_scalar_add(phi[:, sl], pt[:], bk[:])
            for mc in range(MCH):
                msl = slice(mc * P, (mc + 1) * P)
                pt = ps.tile([P, P], F32, tag="ps_o")
                for cb in range(CB):
                    nc.tensor.matmul(
                        pt[:], xb16[cb][:, msl], wvb[:, cb * P:(cb + 1) * P],
                        start=(cb == 0), stop=(cb == CB - 1))
                nc.vector.tensor_add(gT[:, msl], pt[:], bv[:])

            # ---- phase B: attention + out-conv + partial stats ----
            outs = [bigs.tile([P, NQ], F32, name=f"out{cb}", tag=f"out{cb}")
                    for cb in range(CB)]
            s1 = consts.tile([P, CB * NB], F32)   # per-block partial sums
            s2 = consts.tile([P, CB * NB], F32)

            for j in range(NB):
                sl = slice(j * NBLK, (j + 1) * NBLK)
                y_ps = ps.tile([P, NBLK], F32, tag="ps_y")
                r_ps = ps.tile([P, NBLK], F32, tag="ps_r")
                for mc in range(MCH):
                    msl = slice(mc * P, (mc + 1) * P)
                    s_ps = ps.tile([P, NBLK], F32, tag="ps_s")
                    nc.tensor.matmul(s_ps[:], phi[:, msl], theta[:, sl],
                                     start=True, stop=True)
                    pT = ptp.tile([P, NBLK], F16, tag="pT")
                    nc.scalar.activation(pT[:], s_ps[:], AF.Exp, scale=SCALE)
                    nc.tensor.matmul(y_ps[:], gT[:, msl], pT[:],
                                     start=(mc == 0), stop=(mc == MCH - 1))
                    nc.tensor.matmul(r_ps[:], ones[:], pT[:],
                                     start=(mc == 0), stop=(mc == MCH - 1))
                # rho = 1/r via exp(-ln(r)) on ScalarE (joint table set)
                lnr = work.tile([P, NBLK], F32, tag="lnr")
                nc.scalar.activation(lnr[:], r_ps[:], AF.Ln)
                rho = work.tile([P, NBLK], F32, tag="rho")
                nc.scalar.activation(rho[:], lnr[:], AF.Exp, scale=-1.0)
                # unnormalized y to SBUF (f32r) for the out-conv
                ysb = work.tile([P, NBLK], F32R, tag="ysb")
                nc.vector.tensor_copy(ysb[:], y_ps[:])
                for cb in range(CB):
                    o_ps = ps.tile([P, NBLK], F32, tag="ps_o")
                    nc.tensor.matmul(o_ps[:], wo[:, cb * P:(cb + 1) * P],
                                     ysb[:], start=True, stop=True)
                    col = slice(cb * NB + j, cb * NB + j + 1)
                    # normalize + copy out + sum(out) in one DVE op
                    nc.vector.scalar_tensor_tensor(
                        out=outs[cb][:, sl], in0=o_ps[:], scalar=1.0,
                        in1=rho[:], op0=ALU.mult, op1=ALU.mult,
                        accum_out=s1[:, col])
                    sq = work.tile([P, NBLK], F32, tag="sq")
                    nc.vector.scalar_tensor_tensor(
                        out=sq[:], in0=outs[cb][:, sl], scalar=1.0,
                        in1=outs[cb][:, sl], op0=ALU.mult, op1=ALU.mult,
                        accum_out=s2[:, col])

            # ---- phase C: BN stats allreduce + apply + residual ----
            stats = consts.tile([P, 2 * CB], F32)
            for cb in range(CB):
                nc.vector.tensor_reduce(
                    stats[:, cb:cb + 1], s1[:, cb * NB:(cb + 1) * NB],
                    axis=AX.X, op=ALU.add)
                nc.vector.tensor_reduce(
                    stats[:, CB + cb:CB + cb + 1], s2[:, cb * NB:(cb + 1) * NB],
                    axis=AX.X, op=ALU.add)

            cc_in = dram.tile([P, 2 * CB], F32)
            cc_out = dram.tile([P, 2 * CB], F32)
            nc.sync.dma_start(cc_in[:], stats[:])
            nc.gpsimd.collective_compute(
                "AllReduce", ALU.add,
                replica_groups=[list(range(NCORES))],
                ins=[cc_in[:].opt()], outs=[cc_out[:].opt()])
            gstats = consts.tile([P, 2 * CB], F32)
            nc.sync.dma_start(gstats[:], cc_out[:])

            mean = consts.tile([P, CB], F32)
            var = consts.tile([P, CB], F32)
            tmp = consts.tile([P, CB], F32)
            rstd = consts.tile([P, CB], F32)
            a_sc = consts.tile([P, CB], F32)
            b_sc = consts.tile([P, CB], F32)
            nc.vector.tensor_scalar_mul(mean[:], gstats[:, 0:CB], 1.0 / NSAMP)
            nc.vector.tensor_mul(tmp[:], mean[:], mean[:])
            # var = s2/NSAMP - mean^2
            nc.vector.scalar_tensor_tensor(
                out=var[:], in0=gstats[:, CB:2 * CB], scalar=1.0 / NSAMP,
                in1=tmp[:], op0=ALU.mult, op1=ALU.subtract)
            # rstd = exp(-0.5 * ln(var + eps))
            eps_t = consts.tile([P, 1], F32)
            nc.vector.memset(eps_t[:], EPS)
            nc.scalar.activation(tmp[:], var[:], AF.Ln, bias=eps_t[:])
            nc.scalar.activation(rstd[:], tmp[:], AF.Exp, scale=-0.5)
            nc.vector.tensor_mul(a_sc[:], gam[:], rstd[:])
            nc.vector.tensor_mul(tmp[:], a_sc[:], mean[:])
            nc.vector.tensor_sub(b_sc[:], bet[:], tmp[:])

            for cb in range(CB):
                xb = work.tile([P, NQ], F32, tag="xb")
                nc.vector.tensor_scalar_add(xb[:], xs[cb][:, 0:NQ],
                                            b_sc[:, cb:cb + 1])
                for j in range(NB):
                    sl = slice(j * NBLK, (j + 1) * NBLK)
                    f = work.tile([P, NBLK], F32, tag="f")
                    nc.vector.scalar_tensor_tensor(
                        out=f[:], in0=outs[cb][:, sl], scalar=a_sc[:, cb:cb + 1],
                        in1=xb[:, sl], op0=ALU.mult, op1=ALU.add)
                    nc.sync.dma_start(out_d[cb * P:(cb + 1) * P, sl], f[:])

    _compile_with_joint_act_tables(nc)
    return nc


def _get_nc():
    global _CACHED_NC
    if _CACHED_NC is None:
        _CACHED_NC = _build_nc()
    return _CACHED_NC


def _in_maps(inputs):
    x = np.ascontiguousarray(np.asarray(inputs["x"], np.float32)).reshape(B, C, N)
    tw = np.asarray(inputs["theta_w"], np.float32)
    pw = np.asarray(inputs["phi_w"], np.float32)
    gw = np.asarray(inputs["g_w"], np.float32)
    ow = np.asarray(inputs["out_w"], np.float32)

    def pack_ct(w):  # [D, C] -> [128, C] chunk-major transposed
        wt = np.ascontiguousarray(w.T)            # [C, D]
        return np.concatenate([wt[cb * P:(cb + 1) * P, :] for cb in range(CB)],
                              axis=1)             # [P, CB*D]

    wpack = np.concatenate(
        [pack_ct(tw), pack_ct(pw),
         np.ascontiguousarray(ow.T)], axis=1)     # [128, 3*256]
    wvb = pack_ct(gw).astype(np.float16)
    bq = np.asarray(inputs["theta_b"], np.float32).reshape(P, 1)
    bk = np.asarray(inputs["phi_b"], np.float32).reshape(P, 1)
    bv = np.broadcast_to(np.asarray(inputs["g_b"], np.float32)[None, :], (P, P))
    gam = np.asarray(inputs["gamma"], np.float32).reshape(CB, P).T
    bet = np.asarray(inputs["beta"], np.float32).reshape(CB, P).T
    cpack = np.ascontiguousarray(
        np.concatenate([bq, bk, bv, gam, bet], axis=1))  # [128, 134]

    maps = []
    for core in range(NCORES):
        b, h = divmod(core, SPLIT)
        n0 = h * NQ
        xr = x[b] if n0 == 0 else np.ascontiguousarray(
            np.concatenate([x[b][:, n0:], x[b][:, :n0]], axis=1))
        maps.append({"x": xr, "wpack": wpack, "wvb": wvb, "cpack": cpack})
    return maps


def _run(inputs, trace=False, **kw):
    nc = _get_nc()
    maps = _in_maps(inputs)
    r = run_bass_kernel_spmd(nc, maps, list(range(NCORES)), trace=trace, **kw)
    out = np.empty((B, C, N), np.float32)
    for core in range(NCORES):
        b, h = divmod(core, SPLIT)
        out[b][:, h * NQ:(h + 1) * NQ] = r.results[core]["out"]
    return out.reshape(B, C, HGT, WID), r


def kernel(**inputs):
    out, _ = _run(inputs, trace=False)
    return out



# revision 3
# speedup vs baseline: 1.3447x; 1.3447x over previous
"""NonLocalBlock (B=4, C=256, H=W=64) Trainium2 Bass kernel, v2.

Sharding: 8 cores = 4 batch elements x 2 query-row shards of 2048 rows.
Each core receives its batch element's x rotated along N so that its
query rows are columns [0, 2048) -- pure SPMD.

v2 changes vs v1 (217us baseline):
  * fp16 everywhere in attention (theta/phi/g/pT/wo); fp8e4 only for the
    softmax-denominator matmul (r), whose per-element quantization error
    averages out over the 4096-key sum.  Numpy-validated: 2.5e-3 max rel
    vs the 2e-2 gate.
  * r matmul uses fp8 DoubleRow perf mode (2 key-chunks per matmul); the
    all-ones stationary makes the result independent of the HW interleave
    convention.  r matmuls are deferred to a burst at the end of each
    query block so the DVE fp16->fp8 casts are never on the PE critical
    path.
  * EXP runs on [128, 1024] chunk-pairs (one ScalarE op per 2 chunks).
  * rho = 1/r via DVE reciprocal_approx_fast (frees ScalarE, kills the
    Ln/Exp pair per block).
  * Input DMAs via hardware DGE (sync engine) instead of gpsimd SWDGE;
    x arrives in 512-col blocks interleaved with the projections and the
    first attention block so the PE starts early.
  * Dummy 2KB AllReduce at kernel start warms up the ncfw collective
    stack; the real BN-stats AllReduce then runs nearer its ~10us floor
    instead of ~44us.
  * BN apply split ScalarE (a*o+b) + DVE (+x residual, fp16 out); output
    DMA'd as fp16 and upcast on the host.

PSUM budget (8 banks): s pairs 2x[128,1024] (4) + y [128,512] (1) +
r [128,512] (1) + proj/out-conv shared [128,1024] (2).
"""

import math

import numpy as np

import concourse.bass as bass
import concourse.mybir as mybir
import concourse.tile as tile
from concourse import bacc
from concourse.bass_utils import run_bass_kernel_spmd

# Problem constants (hardcoded per contract).
B, C, HGT, WID = 4, 256, 64, 64
N = HGT * WID            # 4096 spatial positions
D = C // 2               # 128 inner channels
P = 128                  # SBUF partitions
NCORES = 8
SPLIT = NCORES // B      # query shards per batch element
NQ = N // SPLIT          # 2048 query rows per core
CB = C // P              # 2 channel chunks
MCH = N // P             # 32 key chunks
NBLK = 512               # query block (one PSUM bank)
NB = NQ // NBLK          # 4 blocks
NPAIR = MCH // 2         # 16 key-chunk pairs per block
EPS = 1e-5
SCALE = 1.0 / math.sqrt(D)
NSAMP = float(B * N)     # BN sample count per channel

F32 = mybir.dt.float32
F16 = mybir.dt.float16
F8 = mybir.dt.float8e4

AF = mybir.ActivationFunctionType
ALU = mybir.AluOpType
AX = mybir.AxisListType
PM = mybir.MatmulPerfMode

_CACHED_NC = None

# cpack column layout: bq | bk | bv_wide(1024) | gamma(2) | beta(2)
CP_BQ = 0
CP_BK = 1
CP_BV = 2
CP_GAM = CP_BV + 1024
CP_BET = CP_GAM + CB
CP_COLS = CP_BET + CB


def _compile_with_joint_act_tables(nc):
    """Run bacc passes with Exp/Ln resolving to the joint table set (avoids
    ~1.3us table reloads when the two alternate)."""
    real = bacc.get_activation_tables

    def patched(arch):
        t = dict(real(arch))
        for k in ("exp_and_others", "natural_log"):
            if k in t:
                t[k] = type(t[k])()
        return t

    bacc.get_activation_tables = patched
    try:
        nc.compile()
    finally:
        bacc.get_activation_tables = real


def _build_nc():
    nc = bacc.Bacc("TRN2", target_bir_lowering=False, debug=False,
                   num_devices=NCORES)

    x_d = nc.dram_tensor("x", [C, N], F32, kind="ExternalInput")
    # fp16 weights: wq_ct | wk_ct (chunk-major transposed) | wo_t
    wp_d = nc.dram_tensor("wpack", [P, 3 * C], F16, kind="ExternalInput")
    wv_d = nc.dram_tensor("wvb", [P, C], F16, kind="ExternalInput")
    cp_d = nc.dram_tensor("cpack", [P, CP_COLS], F32, kind="ExternalInput")
    out_d = nc.dram_tensor("out", [C, NQ], F16, kind="ExternalOutput")

    with tile.TileContext(nc) as tc:
        with (
            tc.tile_pool(name="consts", bufs=1) as consts,
            tc.tile_pool(name="bigs", bufs=1) as bigs,
            tc.tile_pool(name="ptp", bufs=3) as ptp,
            tc.tile_pool(name="ptp8", bufs=NPAIR + 2) as ptp8,
            tc.tile_pool(name="work", bufs=2) as work,
            tc.tile_pool(name="ps", bufs=1, space="PSUM") as ps,
            tc.tile_pool(name="dram", bufs=1, space="DRAM") as dram,
        ):
            # ---- collective warm-up: dummy 2KB AllReduce, result unused ----
            dummy_sb = consts.tile([P, 2 * CB], F32)
            nc.vector.memset(dummy_sb[:], 0.0)
            cc_warm_in = dram.tile([P, 2 * CB], F32)
            cc_warm_out = dram.tile([P, 2 * CB], F32)
            nc.sync.dma_start(cc_warm_in[:], dummy_sb[:])
            nc.gpsimd.collective_compute(
                "AllReduce", ALU.add,
                replica_groups=[list(range(NCORES))],
                ins=[cc_warm_in[:].opt()], outs=[cc_warm_out[:].opt()])

            # ---- constant / weight loads via hw DGE ----
            wpack = consts.tile([P, 3 * C], F16)
            wvb = consts.tile([P, C], F16)
            cpack = consts.tile([P, CP_COLS], F32)
            nc.sync.dma_start(wpack[:], wp_d[:])
            nc.sync.dma_start(wvb[:], wv_d[:])
            nc.sync.dma_start(cpack[:], cp_d[:])
            wq = wpack[:, 0 * C:1 * C]
            wk = wpack[:, 1 * C:2 * C]
            wo = wpack[:, 2 * C:3 * C]
            bq = cpack[:, CP_BQ:CP_BQ + 1]
            bk = cpack[:, CP_BK:CP_BK + 1]
            bv = cpack[:, CP_BV:CP_BV + 1024]
            gam = cpack[:, CP_GAM:CP_GAM + CB]
            bet = cpack[:, CP_BET:CP_BET + CB]
            ones8 = consts.tile([P, C], F8)
            nc.vector.memset(ones8[:], 1.0)
            ones8_dr = ones8[:].rearrange("p (two d) -> p two d", two=2)

            # ---- x load (f32, hw DGE, 512-col blocks) ----
            NXB = N // NBLK  # 8 blocks per channel chunk
            xs = [bigs.tile([P, N], F32, name=f"x{cb}", tag=f"x{cb}")
                  for cb in range(CB)]
            for k in range(NXB):
                ksl = slice(k * NBLK, (k + 1) * NBLK)
                for cb in range(CB):
                    nc.sync.dma_start(xs[cb][:, ksl],
                                      x_d[cb * P:(cb + 1) * P, ksl])
            x16 = [bigs.tile([P, N], F16, name=f"xh{cb}", tag=f"xh{cb}")
                   for cb in range(CB)]

            def cast_block(k):  # x f32 -> fp16 on ScalarE
                ksl = slice(k * NBLK, (k + 1) * NBLK)
                for cb in range(CB):
                    nc.scalar.activation(x16[cb][:, ksl], xs[cb][:, ksl],
                                         AF.Copy)

            theta = bigs.tile([P, NQ], F16, tag="theta")
            phi = bigs.tile([P, N], F16, tag="phi")
            gT = bigs.tile([P, N], F16, tag="gT")  # [m%128, 128*mc + d]

            def proj_theta(t):  # theta blocks 2t, 2t+1
                pt = ps.tile([P, 1024], F32, tag="po", bufs=1, name="pt_t")
                for h in range(2):
                    jsl = slice((2 * t + h) * NBLK, (2 * t + h + 1) * NBLK)
                    for cb in range(CB):
                        nc.tensor.matmul(
                            pt[:, h * NBLK:(h + 1) * NBLK],
                            wq[:, cb * P:(cb + 1) * P], x16[cb][:, jsl],
                            start=(cb == 0), stop=(cb == CB - 1))
                for h in range(2):
                    jsl = slice((2 * t + h) * NBLK, (2 * t + h + 1) * NBLK)
                    nc.vector.tensor_scalar_add(
                        theta[:, jsl], pt[:, h * NBLK:(h + 1) * NBLK], bq[:])

            def proj_phi(t):  # phi blocks 2t, 2t+1
                pt = ps.tile([P, 1024], F32, tag="po", bufs=1, name="pt_p")
                for h in range(2):
                    ksl = slice((2 * t + h) * NBLK, (2 * t + h + 1) * NBLK)
                    for cb in range(CB):
                        nc.tensor.matmul(
                            pt[:, h * NBLK:(h + 1) * NBLK],
                            wk[:, cb * P:(cb + 1) * P], x16[cb][:, ksl],
                            start=(cb == 0), stop=(cb == CB - 1))
                for h in range(2):
                    ksl = slice((2 * t + h) * NBLK, (2 * t + h + 1) * NBLK)
                    nc.vector.tensor_scalar_add(
                        phi[:, ksl], pt[:, h * NBLK:(h + 1) * NBLK], bk[:])

            def proj_g(t):  # gT chunks 8t .. 8t+7
                pt = ps.tile([P, 1024], F32, tag="po", bufs=1, name="pt_g")
                for q in range(8):
                    msl = slice((8 * t + q) * P, (8 * t + q + 1) * P)
                    for cb in range(CB):
                        nc.tensor.matmul(
                            pt[:, q * P:(q + 1) * P],
                            x16[cb][:, msl], wvb[:, cb * P:(cb + 1) * P],
                            start=(cb == 0), stop=(cb == CB - 1))
                gsl = slice(8 * t * P, 8 * (t + 1) * P)
                nc.vector.tensor_add(gT[:, gsl], pt[:], bv[:])

            # ---- phase B state ----
            outs = [bigs.tile([P, NQ], F32, name=f"out{cb}", tag=f"out{cb}")
                    for cb in range(CB)]
            s1 = consts.tile([P, CB * NB], F32)
            s2 = consts.tile([P, CB * NB], F32)

            def attn_pairs(j, pcs, y_ps):
                """S + EXP + pT8-cast + y for pairs `pcs` of block j."""
                jsl = slice(j * NBLK, (j + 1) * NBLK)
                res = []
                for pc in pcs:
                    s_ps = ps.tile([P, 1024], F32, tag="ps_s", bufs=2,
                                   name="s_ps")
                    for h in range(2):
                        msl = slice((2 * pc + h) * P, (2 * pc + h + 1) * P)
                        nc.tensor.matmul(s_ps[:, h * NBLK:(h + 1) * NBLK],
                                         phi[:, msl], theta[:, jsl],
                                         start=True, stop=True)
                    pT16 = ptp.tile([P, 1024], F16, tag="pT16", name="pT16")
                    nc.scalar.activation(pT16[:], s_ps[:], AF.Exp, scale=SCALE)
                    pT8 = ptp8.tile([P, 1024], F8, tag="pT8", name="pT8")
                    with nc.allow_low_precision("fp8 softmax denominator"):
                        nc.vector.tensor_copy(pT8[:], pT16[:])
                    for h in range(2):
                        msl = slice((2 * pc + h) * P, (2 * pc + h + 1) * P)
                        nc.tensor.matmul(
                            y_ps[:], gT[:, msl],
                            pT16[:, h * NBLK:(h + 1) * NBLK],
                            start=(pc == 0 and h == 0),
                            stop=(pc == NPAIR - 1 and h == 1))
                    res.append(pT8)
                return res

            def block_tail(j, y_ps, pT8s):
                """r-burst, rho, out-conv, stats for block j."""
                jsl = slice(j * NBLK, (j + 1) * NBLK)
                r_ps = ps.tile([P, NBLK], F32, tag="ps_r", bufs=1, name="r_ps")
                for i, pT8 in enumerate(pT8s):
                    nc.tensor.matmul(
                        r_ps[:], ones8_dr,
                        pT8[:].rearrange("p (two n) -> p two n", two=2),
                        start=(i == 0), stop=(i == len(pT8s) - 1),
                        perf_mode=PM.DoubleRow)
                rho = work.tile([P, NBLK], F32, tag="rho", name="rho")
                nc.vector.reciprocal_approx_fast(rho[:], r_ps[:])
                ysb = work.tile([P, NBLK], F16, tag="ysb", name="ysb")
                with nc.allow_low_precision("fp16 attention numerator"):
                    nc.vector.tensor_copy(ysb[:], y_ps[:])
                o_ps = ps.tile([P, 1024], F32, tag="po", bufs=1, name="o_ps")
                for cb in range(CB):
                    nc.tensor.matmul(o_ps[:, cb * NBLK:(cb + 1) * NBLK],
                                     wo[:, cb * P:(cb + 1) * P], ysb[:],
                                     start=True, stop=True)
                for cb in range(CB):
                    col = slice(cb * NB + j, cb * NB + j + 1)
                    nc.vector.scalar_tensor_tensor(
                        out=outs[cb][:, jsl],
                        in0=o_ps[:, cb * NBLK:(cb + 1) * NBLK], scalar=1.0,
                        in1=rho[:], op0=ALU.mult, op1=ALU.mult,
                        accum_out=s1[:, col])
                    sq = work.tile([P, NBLK], F32, tag="sq", name="sq")
                    nc.vector.scalar_tensor_tensor(
                        out=sq[:], in0=outs[cb][:, jsl], scalar=1.0,
                        in1=outs[cb][:, jsl], op0=ALU.mult, op1=ALU.mult,
                        accum_out=s2[:, col])

            # ---- phase A/B interleaved schedule ----
            cast_block(0)
            cast_block(1)
            proj_theta(0)
            proj_phi(0)
            proj_g(0)
            cast_block(2)
            cast_block(3)
            proj_phi(1)
            proj_g(1)
            y_ps0 = ps.tile([P, NBLK], F32, tag="ps_y", bufs=1, name="y_ps")
            pT8s0 = attn_pairs(0, list(range(0, 4)), y_ps0)
            cast_block(4)
            cast_block(5)
            proj_theta(1)
            proj_phi(2)
            proj_g(2)
            pT8s0 += attn_pairs(0, list(range(4, 8)), y_ps0)
            cast_block(6)
            cast_block(7)
            proj_phi(3)
            proj_g(3)
            pT8s0 += attn_pairs(0, list(range(8, NPAIR)), y_ps0)
            block_tail(0, y_ps0, pT8s0)
            for j in range(1, NB):
                y_psj = ps.tile([P, NBLK], F32, tag="ps_y", bufs=1,
                                name="y_ps")
                pT8s = attn_pairs(j, list(range(NPAIR)), y_psj)
                block_tail(j, y_psj, pT8s)

            # ---- phase C: BN stats allreduce + apply + residual ----
            stats = consts.tile([P, 2 * CB], F32)
            for cb in range(CB):
                nc.vector.tensor_reduce(
                    stats[:, cb:cb + 1], s1[:, cb * NB:(cb + 1) * NB],
                    axis=AX.X, op=ALU.add)
                nc.vector.tensor_reduce(
                    stats[:, CB + cb:CB + cb + 1], s2[:, cb * NB:(cb + 1) * NB],
                    axis=AX.X, op=ALU.add)

            cc_in = dram.tile([P, 2 * CB], F32)
            cc_out = dram.tile([P, 2 * CB], F32)
            nc.sync.dma_start(cc_in[:], stats[:])
            nc.gpsimd.collective_compute(
                "AllReduce", ALU.add,
                replica_groups=[list(range(NCORES))],
                ins=[cc_in[:].opt()], outs=[cc_out[:].opt()])
            gstats = consts.tile([P, 2 * CB], F32)
            nc.sync.dma_start(gstats[:], cc_out[:])

            mean = consts.tile([P, CB], F32)
            var = consts.tile([P, CB], F32)
            tmp = consts.tile([P, CB], F32)
            rstd = consts.tile([P, CB], F32)
            a_sc = consts.tile([P, CB], F32)
            b_sc = consts.tile([P, CB], F32)
            nc.vector.tensor_scalar_mul(mean[:], gstats[:, 0:CB], 1.0 / NSAMP)
            nc.vector.tensor_mul(tmp[:], mean[:], mean[:])
            nc.vector.scalar_tensor_tensor(
                out=var[:], in0=gstats[:, CB:2 * CB], scalar=1.0 / NSAMP,
                in1=tmp[:], op0=ALU.mult, op1=ALU.subtract)
            # rstd = exp(-0.5 * ln(var + eps))
            eps_t = consts.tile([P, 1], F32)
            nc.vector.memset(eps_t[:], EPS)
            nc.scalar.activation(tmp[:], var[:], AF.Ln, bias=eps_t[:])
            nc.scalar.activation(rstd[:], tmp[:], AF.Exp, scale=-0.5)
            nc.vector.tensor_mul(a_sc[:], gam[:], rstd[:])
            nc.vector.tensor_mul(tmp[:], a_sc[:], mean[:])
            nc.vector.tensor_sub(b_sc[:], bet[:], tmp[:])

            # apply: t = a*o + b (ScalarE), f = t + x fp16 (DVE), DMA out
            GRP = 1024
            for cb in range(CB):
                for g0 in range(NQ // GRP):
                    gsl = slice(g0 * GRP, (g0 + 1) * GRP)
                    t = work.tile([P, GRP], F32, tag="t_apply", name="t_ap")
                    nc.scalar.activation(t[:], outs[cb][:, gsl], AF.Identity,
                                         bias=b_sc[:, cb:cb + 1],
                                         scale=a_sc[:, cb:cb + 1])
                    f = work.tile([P, GRP], F16, tag="f_apply", name="f_ap")
                    with nc.allow_low_precision("fp16 output"):
                        nc.vector.tensor_add(f[:], t[:], xs[cb][:, gsl])
                    nc.sync.dma_start(out_d[cb * P:(cb + 1) * P, gsl], f[:])

    _compile_with_joint_act_tables(nc)
    return nc


def _get_nc():
    global _CACHED_NC
    if _CACHED_NC is None:
        _CACHED_NC = _build_nc()
    return _CACHED_NC


def _in_maps(inputs):
    x = np.ascontiguousarray(np.asarray(inputs["x"], np.float32)).reshape(B, C, N)
    tw = np.asarray(inputs["theta_w"], np.float32)
    pw = np.asarray(inputs["phi_w"], np.float32)
    gw = np.asarray(inputs["g_w"], np.float32)
    ow = np.asarray(inputs["out_w"], np.float32)

    def pack_ct(w):  # [D, C] -> [128, C] chunk-major transposed
        wt = np.ascontiguousarray(w.T)            # [C, D]
        return np.concatenate([wt[cb * P:(cb + 1) * P, :] for cb in range(CB)],
                              axis=1)             # [P, CB*D]

    wpack = np.concatenate(
        [pack_ct(tw), pack_ct(pw),
         np.ascontiguousarray(ow.T)], axis=1).astype(np.float16)
    wvb = pack_ct(gw).astype(np.float16)
    bq = np.asarray(inputs["theta_b"], np.float32).reshape(P, 1)
    bk = np.asarray(inputs["phi_b"], np.float32).reshape(P, 1)
    bv = np.broadcast_to(np.asarray(inputs["g_b"], np.float32)[None, :], (P, P))
    bv_wide = np.tile(bv, (1, 8))                 # [128, 1024]
    gam = np.asarray(inputs["gamma"], np.float32).reshape(CB, P).T
    bet = np.asarray(inputs["beta"], np.float32).reshape(CB, P).T
    cpack = np.ascontiguousarray(
        np.concatenate([bq, bk, bv_wide, gam, bet], axis=1))

    maps = []
    for core in range(NCORES):
        b, h = divmod(core, SPLIT)
        n0 = h * NQ
        xr = x[b] if n0 == 0 else np.ascontiguousarray(
            np.concatenate([x[b][:, n0:], x[b][:, :n0]], axis=1))
        maps.append({"x": xr, "wpack": wpack, "wvb": wvb, "cpack": cpack})
    return maps


def _run(inputs, trace=False, **kw):
    nc = _get_nc()
    maps = _in_maps(inputs)
    r = run_bass_kernel_spmd(nc, maps, list(range(NCORES)), trace=trace, **kw)
    out = np.empty((B, C, N), np.float32)
    for core in range(NCORES):
        b, h = divmod(core, SPLIT)
        out[b][:, h * NQ:(h + 1) * NQ] = r.results[core]["out"].astype(np.float32)
    return out.reshape(B, C, HGT, WID), r


def kernel(**inputs):
    out, _ = _run(inputs, trace=False)
    return out
